# revision 1
# baseline (speedup 1.0000x reference)
"""ContextualLoss forward for Trainium2 (8 NeuronCores, Bass/Tile).

Math (per batch b):
  mu    = mean over spatial of Y[b]                     [C]
  Xc/Yc = centered features                             [C, N]
  Xn/Yn = L2-normalized over C (eps ~ 2.2e-16, negligible in fp32)
  S     = Xn.T @ Yn (cosine similarity)                 [N, N]
  d     = 1 - S  (relu clamp never triggers for randn data, |S| << 1)
  m_i   = min_j d_ij = 1 - max_j S_ij
  A_ij  = softmax_j( -d_ij / (h*(m_i + 1e-5)) )   (the exp(1/h) factor cancels)
  CX_b  = mean_i max_j A_ij = mean_i exp(-alpha_i*m_i) / sum_j exp(-alpha_i*d_ij)
  loss  = mean_b( -log CX_b )

Sharding: 8 cores = 4 batches x 2 row-halves. Each core takes its X column
slice [C, 2048] plus the full Y[b] [C, 4096], computes the row-wise softmax
stats for its 2048 rows, and returns sum_i max_j A_ij as one scalar. The
final tiny reduction (8 scalars -> loss) happens on host in float64.

Per-core pipeline (i-chunks of 128 rows):
  PE   : S chunk [128, 2048] into PSUM (bf16 inputs, fp32 accumulate)
  DVE  : tensor_tensor_reduce: copy S -> SBUF bf16 + fused row-max
  ACT  : one exp over [128, 2*2048+1] with per-partition scale/bias
         (column 4096 holds the raw row max, so its exp is the softmax
         numerator exp(-alpha*m) for free) + fused row-sum
  DVE  : tiny per-row chain (alpha, numerator/denominator, accumulate)
"""

import numpy as np

import bass_rust
import concourse.bacc as bacc
import concourse.bass as bass
import concourse.tile as tile
from concourse import mybir
from concourse.alu_op_type import AluOpType
from concourse.bass_utils import run_bass_kernel_spmd

P = 128          # partitions
C = 256          # channels
N = 4096         # spatial positions (64*64)
NH = N // 2      # rows per core (row-half)
KC = C // P      # channel chunks (2)
NCHUNK = NH // P # i-chunks per core (16)
JW = 2048        # j half width (PSUM tile free size)
NJH = N // JW    # j halves (2)
MMW = 512        # matmul free width (one PSUM bank of fp32)
H_BW = 0.1
F32 = mybir.dt.float32
BF16 = mybir.dt.bfloat16
NEG_BIG = -3.38e38


def _build_program(nc: bass.Bass):
    x = nc.dram_tensor("x", [C, NH], F32, kind="ExternalInput")
    y = nc.dram_tensor("y", [C, N], F32, kind="ExternalInput")
    out = nc.dram_tensor("out", [1, 1], F32, kind="ExternalOutput")

    xv = x.ap().rearrange("(k p) n -> p k n", p=P)   # [128, 2, 2048]
    yv = y.ap().rearrange("(k p) n -> p k n", p=P)   # [128, 2, 4096]

    with tile.TileContext(nc) as tc:
        with (
            tc.tile_pool(name="singles", bufs=1) as singles,
            tc.tile_pool(name="rowstat", bufs=4) as rowstat,
            tc.tile_pool(name="scp_pool", bufs=3) as scp_pool,
        ):
            # ---------------- load ----------------
            # Y in two j-half tiles so the mean accumulation can start after
            # the first half lands.
            QW = 1024            # j-quarter width (matches main-loop PSUM tiles)
            NQ = N // QW
            yth = [singles.tile([P, KC, N // 2], F32, tag=f"yt{h}", name=f"yt{h}") for h in range(2)]
            for h in range(2):
                nc.sync.dma_start(
                    out=yth[h][:], in_=yv[:, :, h * (N // 2) : (h + 1) * (N // 2)]
                )
            xt = singles.tile([P, KC, NH], F32, tag="xt")
            nc.sync.dma_start(out=xt[:], in_=xv)

            ones_col = singles.tile([P, 1], BF16, tag="ones_col")
            nc.vector.memset(ones_col[:], 1.0)

            # ---------------- preprocessing ----------------
            # mu (mean of Y over spatial) via ACT copy with fused row-sum
            mus = rowstat.tile([P, 2 * KC], F32, tag="mus")
            muscr = singles.tile([P, N // 2], BF16, tag="muscr")
            for h in range(2):
                for k in range(KC):
                    nc.scalar.activation(
                        out=muscr[:], in_=yth[h][:, k, :],
                        func=mybir.ActivationFunctionType.Copy,
                        accum_out=mus[:, 2 * h + k : 2 * h + k + 1],
                    )
            negmu = singles.tile([P, KC], F32, tag="negmu")
            for k in range(KC):
                nc.vector.tensor_tensor(
                    out=negmu[:, k : k + 1], in0=mus[:, k : k + 1],
                    in1=mus[:, 2 + k : 3 + k], op=AluOpType.add,
                )
            nc.vector.tensor_scalar(
                out=negmu[:], in0=negmu[:], scalar1=-1.0 / N, scalar2=None,
                op0=AluOpType.mult,
            )

            # Y chain per j-quarter (separate tiles => the main loop's first
            # matmuls only wait on quarter 0's chain, not all preprocessing):
            # center+cast -> square -> channel-sumsq (ones matmul) -> sqrt ->
            # reciprocal -> partition broadcast -> scale Yc by 1/||Y_j||
            ycq = [
                singles.tile([P, KC, QW], BF16, tag=f"ycq{q}", name=f"ycq{q}")
                for q in range(NQ)
            ]
            with tc.tile_pool(name="pre_psum", bufs=1, space="PSUM") as pre_psum:
                for q in range(NQ):
                    h, hoff = q // 2, (q % 2) * QW
                    for k in range(KC):
                        nc.vector.tensor_scalar(
                            out=ycq[q][:, k, :],
                            in0=yth[h][:, k, hoff : hoff + QW],
                            scalar1=negmu[:, k : k + 1], scalar2=None,
                            op0=AluOpType.add,
                        )
                    ysq = scp_pool.tile([P, KC, QW], BF16, tag="ysq")
                    for k in range(KC):
                        nc.vector.tensor_tensor(
                            out=ysq[:, k, :], in0=ycq[q][:, k, :],
                            in1=ycq[q][:, k, :], op=AluOpType.mult,
                        )
                    ssq = pre_psum.tile([1, QW], F32, tag=f"ssq{q}")
                    for w in range(QW // MMW):
                        for k in range(KC):
                            nc.tensor.matmul(
                                ssq[:, w * MMW : (w + 1) * MMW],
                                ones_col[:],
                                ysq[:, k, w * MMW : (w + 1) * MMW],
                                start=(k == 0),
                                stop=(k == KC - 1),
                            )
                    scy = rowstat.tile([1, QW], F32, tag="scy")
                    nc.scalar.activation(
                        out=scy[:], in_=ssq[:],
                        func=mybir.ActivationFunctionType.Sqrt,
                    )
                    cinv = rowstat.tile([1, QW], BF16, tag="cinv")
                    with nc.allow_low_precision(reason="bf16 scale for bf16 matmul"):
                        nc.vector.reciprocal(out=cinv[:], in_=scy[:])
                    cbro = scp_pool.tile([P, QW], BF16, tag="cbro")
                    nc.gpsimd.partition_broadcast(cbro[:], cinv[:])
                    for k in range(KC):
                        nc.vector.tensor_tensor(
                            out=ycq[q][:, k, :], in0=ycq[q][:, k, :],
                            in1=cbro[:], op=AluOpType.mult,
                        )

            # X side: center+cast, squares, per-i-block channel sumsq via
            # matmul with ones as the moving operand -> rinv in [128, 16]
            xcbf = singles.tile([P, KC, NH], BF16, tag="xcbf")
            xsq = singles.tile([P, KC, NH], BF16, tag="xsq")
            for k in range(KC):
                nc.vector.tensor_scalar(
                    out=xcbf[:, k, :], in0=xt[:, k, :],
                    scalar1=negmu[:, k : k + 1], scalar2=None, op0=AluOpType.add,
                )
                nc.scalar.activation(
                    out=xsq[:, k, :], in_=xcbf[:, k, :],
                    func=mybir.ActivationFunctionType.Square,
                )
            with tc.tile_pool(name="prex_psum", bufs=1, space="PSUM") as prex_psum:
                ssq_x = prex_psum.tile([P, NCHUNK], F32, tag="ssq_x")
                for blk in range(NCHUNK):
                    for k in range(KC):
                        nc.tensor.matmul(
                            ssq_x[:, blk : blk + 1],
                            xsq[:, k, blk * P : (blk + 1) * P],
                            ones_col[:],
                            start=(k == 0),
                            stop=(k == KC - 1),
                        )
                srx = singles.tile([P, NCHUNK], F32, tag="srx")
                nc.scalar.activation(
                    out=srx[:], in_=ssq_x[:], func=mybir.ActivationFunctionType.Sqrt,
                )
            rinv = singles.tile([P, NCHUNK], F32, tag="rinv")
            nc.vector.reciprocal(out=rinv[:], in_=srx[:])
            neg_rinv = singles.tile([P, NCHUNK], F32, tag="neg_rinv")
            nc.vector.tensor_scalar(
                out=neg_rinv[:], in0=rinv[:], scalar1=-1.0, scalar2=None,
                op0=AluOpType.mult,
            )

            # ---------------- main loop ----------------
            # PSUM quarters [128, QW] (2 banks each, 4 in flight). DVE row-max
            # reads each quarter as soon as its matmuls land; ACT's exp (with
            # fused row-sum) is the last PSUM reader and frees the slot for
            # the next chunk's matmuls.
            numt = singles.tile([P, NCHUNK], F32, tag="numt")
            sumet = singles.tile([P, NCHUNK], F32, tag="sumet")
            with tc.tile_pool(name="mm_psum", bufs=4, space="PSUM") as mm_psum:
                for ch in range(NCHUNK):
                    scp = scp_pool.tile([P, N + 8], BF16, tag="scp")
                    for q in range(NQ):
                        smat = mm_psum.tile([P, QW], F32, tag="smat")
                        for k in range(KC):
                            lhsT = xcbf[:, k, ch * P : (ch + 1) * P]
                            for w in range(QW // MMW):
                                nc.tensor.matmul(
                                    smat[:, w * MMW : (w + 1) * MMW],
                                    lhsT,
                                    ycq[q][:, k, w * MMW : (w + 1) * MMW],
                                    start=(k == 0),
                                    stop=(k == KC - 1),
                                )
                        # drain PSUM to bf16 SBUF right away (not gated on
                        # the row max), alternating DVE/ACT to balance load
                        dst = scp[:, q * QW : (q + 1) * QW]
                        if q % 2 == 0:
                            with nc.allow_low_precision(reason="S copy bf16"):
                                nc.vector.tensor_copy(out=dst, in_=smat[:])
                        else:
                            nc.scalar.copy(out=dst, in_=smat[:])

                    # row max over the whole bf16 S chunk
                    mx = rowstat.tile([P, 1], F32, tag="mx")
                    nc.vector.reduce_max(
                        out=mx[:], in_=scp[:, :N], axis=mybir.AxisListType.X,
                    )
                    # stash raw row max as column N: its exp is the numerator
                    with nc.allow_low_precision(reason="numerator column bf16"):
                        nc.vector.tensor_copy(out=scp[:, N : N + 1], in_=mx[:])
                    # m = relu(1 - rinv*mx)
                    m_t = rowstat.tile([P, 1], F32, tag="m_t")
                    nc.vector.tensor_scalar(
                        out=m_t[:], in0=mx[:],
                        scalar1=neg_rinv[:, ch : ch + 1], scalar2=1.0,
                        op0=AluOpType.mult, op1=AluOpType.add,
                    )
                    nc.vector.tensor_scalar_max(m_t[:], m_t[:], 0.0)
                    # negalpha = -1 / (h*(m + 1e-5)) = 1 / (-h*m - h*1e-5)
                    negal = rowstat.tile([P, 1], F32, tag="negal")
                    nc.vector.tensor_scalar(
                        out=negal[:], in0=m_t[:],
                        scalar1=-H_BW, scalar2=-H_BW * 1e-5,
                        op0=AluOpType.mult, op1=AluOpType.add,
                    )
                    nc.vector.reciprocal(out=negal[:], in_=negal[:])
                    # scale_i = alpha * rinv = negalpha * neg_rinv
                    scl = rowstat.tile([P, 1], F32, tag="scl")
                    nc.vector.tensor_tensor(
                        out=scl[:], in0=negal[:], in1=neg_rinv[:, ch : ch + 1],
                        op=AluOpType.mult,
                    )
                    # one exp over S + numerator column, fused row-sum
                    nc.scalar.activation(
                        out=scp[:, : N + 1], in_=scp[:, : N + 1],
                        func=mybir.ActivationFunctionType.Exp,
                        bias=negal[:], scale=scl[:],
                        accum_out=sumet[:, ch : ch + 1],
                    )
                    # extract numerator on ACT (post-exp, keeps DVE unstalled)
                    nc.scalar.copy(out=numt[:, ch : ch + 1], in_=scp[:, N : N + 1])

            # batched epilogue: rowA = num / (sumexp - num) for all chunks
            den = singles.tile([P, NCHUNK], F32, tag="den")
            nc.vector.tensor_tensor(
                out=den[:], in0=sumet[:], in1=numt[:], op=AluOpType.subtract,
            )
            rden = singles.tile([P, NCHUNK], F32, tag="rden")
            nc.vector.reciprocal(out=rden[:], in_=den[:])
            res = singles.tile([P, NCHUNK], F32, tag="res")
            nc.vector.tensor_tensor(
                out=res[:], in0=numt[:], in1=rden[:], op=AluOpType.mult,
            )

            # ---------------- final reduction ----------------
            rsum = singles.tile([P, 1], F32, tag="rsum")
            nc.vector.reduce_sum(
                out=rsum[:], in_=res[:], axis=mybir.AxisListType.X,
            )
            tot = singles.tile([P, 1], F32, tag="tot")
            nc.gpsimd.partition_all_reduce(
                tot[:], rsum[:], channels=P, reduce_op=bass_rust.ReduceOp.add,
            )
            nc.sync.dma_start(out=out.ap(), in_=tot[0:1, :])

    return nc


_NC_CACHE = None


def _get_nc():
    global _NC_CACHE
    if _NC_CACHE is None:
        nc = bacc.Bacc("TRN2", target_bir_lowering=False, debug=False)
        _build_program(nc)
        nc.compile()
        _NC_CACHE = nc
    return _NC_CACHE


def kernel(X_features: np.ndarray, Y_features: np.ndarray) -> np.ndarray:
    B = X_features.shape[0]
    X = np.ascontiguousarray(X_features, dtype=np.float32).reshape(B, C, N)
    Y = np.ascontiguousarray(Y_features, dtype=np.float32).reshape(B, C, N)

    in_maps = []
    for core in range(8):
        b, h = core // 2, core % 2
        in_maps.append(
            {
                "x": np.ascontiguousarray(X[b][:, h * NH : (h + 1) * NH]),
                "y": Y[b],
            }
        )

    nc = _get_nc()
    results = run_bass_kernel_spmd(nc, in_maps, list(range(8))).results
    sums = np.array(
        [results[core]["out"][0, 0] for core in range(8)], dtype=np.float64
    )
    cx = sums.reshape(B, 2).sum(axis=1) / float(N)
    loss = float(np.mean(-np.log(cx)))
    return np.asarray(loss, dtype=np.float32)

